# revision 1
# baseline (speedup 1.0000x reference)
"""Trainium2 Bass kernel for AttentionWithRoPE (B=2, S=2048, HID=2048, H=16, D=128).

Sharding (8 cores): tensor-parallel over heads x data-parallel over batch.
Core c handles batch c//4 and heads 4*(c%4) .. 4*(c%4)+4:
  - QKV^T projections as fp32r PE matmuls with the HID contraction on
    partitions (hidden arrives pre-transposed from the host). Q pass, K pass,
    then V pass; Q^T/K^T/V stay resident in SBUF (a 2-slot rotating pool
    hosts wq/wk -> wv -> A^T over time, so each phase's loads prefetch
    during the previous one). RoPE is fused on the DVE reading the
    projection PSUM directly (partition-shifted reads are legal vs PSUM).
  - Causal attention per head in scores^T orientation ([k, q]: the PV
    contraction dim k lands on partitions, so P^T feeds the PE directly and
    softmax needs no transposes). exp on ScalarE straight from PSUM with the
    1/sqrt(D) scale folded in; fully-masked k-blocks are skipped; diagonal
    blocks multiply a 0/1 upper-tri mask; the softmax denominator is a
    ones-vector PE matmul accumulated alongside PV; normalization is
    reciprocal + K=1 PE-matmul broadcast + DVE multiply.
  - Output projection partial with the core's w_o column slice; the host
    sums the four partials per batch (the TP reduce).
All matmul operands are float32r (TF32-like: full PE rate at moving
free-dim >= 256, ~1.5e-4 rel err); end-to-end output error vs the fp32
reference is ~2e-4. Big DMAs are chunked so consumers start on the first
chunk; small DMAs are merged to amortize descriptor cost.
"""
import numpy as np
from contextlib import ExitStack

import concourse.bass as bass
import concourse.tile as tile
from concourse import bacc, mybir
from concourse.bass_utils import run_bass_kernel_spmd

B, S, HID = 2, 2048, 2048
H, D = 16, 128
NCORES = 8
NH = 4                 # heads per core
HC = HID // 128        # hid chunks
ST = 256               # phase-A s-tile width
NST = S // ST
QT = 512               # phase-B q-tile width
NQT = S // QT
DSCALE = float(D) ** -0.5
F32 = mybir.dt.float32
F32R = mybir.dt.float32r

_CACHED = {}


def _build_nc():
    nc = bacc.Bacc("TRN2", target_bir_lowering=False, debug=False,
                   num_devices=NCORES)
    hT = nc.dram_tensor("hT", [HID, S], F32R, kind="ExternalInput")
    wqT = nc.dram_tensor("wqT", [HID, NH * D], F32R, kind="ExternalInput")
    wkT = nc.dram_tensor("wkT", [HID, NH * D], F32R, kind="ExternalInput")
    wvT = nc.dram_tensor("wvT", [HID, NH * D], F32R, kind="ExternalInput")
    woT = nc.dram_tensor("woT", [NH * D, HID], F32R, kind="ExternalInput")
    cosT = nc.dram_tensor("cosT", [D, S], F32, kind="ExternalInput")
    sinS = nc.dram_tensor("sinS", [D, S], F32, kind="ExternalInput")
    tri = nc.dram_tensor("tri", [128, 128], F32, kind="ExternalInput")
    ones = nc.dram_tensor("ones", [128, 1], F32R, kind="ExternalInput")
    onesr = nc.dram_tensor("onesr", [1, 128], F32R, kind="ExternalInput")
    out = nc.dram_tensor("out", [S, HID], F32, kind="ExternalOutput")

    hT_r = hT.ap().rearrange("(hc p) s -> p hc s", p=128)
    wqT_r = wqT.ap().rearrange("(hc p) m -> p hc m", p=128)
    wkT_r = wkT.ap().rearrange("(hc p) m -> p hc m", p=128)
    wvT_r = wvT.ap().rearrange("(hc p) m -> p hc m", p=128)
    woT_r = woT.ap().rearrange("(g p) n -> p g n", p=128)

    AST = 512              # phase-A s-tile width (N of the QK-pass matmuls)
    ANST = S // AST

    with tile.TileContext(nc) as tc, ExitStack() as ctx:
        # ---- small constants ----
        constp = ctx.enter_context(tc.tile_pool(name="const", bufs=1))
        tri_sb = constp.tile([128, 128], F32, tag="tri", name="tri")
        nc.sync.dma_start(tri_sb[:], tri.ap())
        ones_sb = constp.tile([128, 1], F32R, tag="ones", name="ones")
        nc.sync.dma_start(ones_sb[:], ones.ap())
        onesr_sb = constp.tile([1, 128], F32R, tag="onesr", name="onesr")
        nc.sync.dma_start(onesr_sb[:], onesr.ap())

        # Q^T/K^T stay resident in SBUF through attention
        qkp = ctx.enter_context(tc.tile_pool(name="qk", bufs=1))
        qsb = qkp.tile([128, NH, S], F32R, tag="qsb", name="qsb")
        ksb = qkp.tile([128, NH, S], F32R, tag="ksb", name="ksb")

        # Rotating 2-slot pool (32KB/partition each) hosting, over time:
        #   slot0: wq -> wv -> at   slot1: wk -> v_sb
        # Tile's WAR tracking turns each reuse into a prefetch window.
        wpool = ctx.enter_context(tc.tile_pool(name="aw", bufs=2))
        wq_sb = wpool.tile([128, HC, NH * D], F32R, tag="w", name="wq")
        wk_sb = wpool.tile([128, HC, NH * D], F32R, tag="w", name="wk")
        for c in range(4):
            h4 = slice(4 * c, 4 * c + 4)
            nc.sync.dma_start(wq_sb[:, h4, :], wqT_r[:, h4, :])
        for c in range(4):
            h4 = slice(4 * c, 4 * c + 4)
            nc.sync.dma_start(wk_sb[:, h4, :], wkT_r[:, h4, :])

        # ================= Phase A: Q pass, K pass =================
        with ExitStack() as astack:
            hpool = astack.enter_context(tc.tile_pool(name="ah", bufs=3))
            cspool = astack.enter_context(tc.tile_pool(name="acs", bufs=2))
            ropep = astack.enter_context(tc.tile_pool(name="arope", bufs=1))
            psA = astack.enter_context(
                tc.tile_pool(name="apsqk", bufs=5, space="PSUM"))

            for st in range(ANST):
                sl = bass.ts(st, AST)
                hb = [hpool.tile([128, HC // 2, AST], F32R, tag="h",
                                 name=f"hb{half}") for half in range(2)]
                for half in range(2):
                    for c in range(4):
                        nc.sync.dma_start(
                            hb[half][:, 2 * c:2 * c + 2, :],
                            hT_r[:, slice(8 * half + 2 * c,
                                          8 * half + 2 * c + 2), sl])
                cs_t = cspool.tile([128, AST], F32, tag="cs", name="cs")
                nc.sync.dma_start(cs_t[:], cosT.ap()[:, sl])
                ss_t = cspool.tile([128, AST], F32, tag="ss", name="ss")
                nc.sync.dma_start(ss_t[:], sinS.ap()[:, sl])
                for wsb, dsb in ((wq_sb, qsb), (wk_sb, ksb)):
                    for h in range(NH):
                        ps = psA.tile([128, AST], F32, tag="psqk",
                                      name="psqk")
                        for hc in range(HC):
                            nc.tensor.matmul(
                                ps[:],
                                wsb[:, hc, h * D:(h + 1) * D],
                                hb[hc // 8][:, hc % 8, :],
                                start=(hc == 0), stop=(hc == HC - 1),
                            )
                        # RoPE: out = x*cos + shift(x)*sin_signed. The
                        # partition-shifted reads go straight to PSUM (walrus
                        # requires equal base partitions only when BOTH
                        # operands are in SBUF).
                        tsin = ropep.tile([128, AST], F32, tag="tsin",
                                          name="tsin")
                        nc.vector.tensor_tensor(
                            tsin[0:64, :], ps[64:128, :], ss_t[0:64, :],
                            mybir.AluOpType.mult)
                        nc.vector.tensor_tensor(
                            tsin[64:128, :], ps[0:64, :], ss_t[64:128, :],
                            mybir.AluOpType.mult)
                        tcos = ropep.tile([128, AST], F32, tag="tcos",
                                          name="tcos")
                        nc.vector.tensor_tensor(
                            tcos[:], ps[:], cs_t[:], mybir.AluOpType.mult)
                        nc.vector.tensor_tensor(
                            dsb[:, h, sl], tcos[:], tsin[:],
                            mybir.AluOpType.add)

            # wv reuses wq's slot; its loads overlap the tail of the QK pass
            wv_sb = wpool.tile([128, HC, NH * D], F32R, tag="w", name="wv")
            for c in range(4):
                h4 = slice(4 * c, 4 * c + 4)
                nc.sync.dma_start(wv_sb[:, h4, :], wvT_r[:, h4, :])

        # ================= Phase A2: V projection =================
        # v_sb reuses wk's slot; natural orientation, resident through B
        v_sb = wpool.tile([128, S // 128, NH * D], F32R, tag="w", name="vsb")
        with ExitStack() as a2ctx:
            h2pool = a2ctx.enter_context(tc.tile_pool(name="ah2", bufs=4))
            psAv = a2ctx.enter_context(
                tc.tile_pool(name="apsv", bufs=3, space="PSUM"))
            for st in range(NST):
                sl = bass.ts(st, ST)
                hq = [h2pool.tile([128, 4, ST], F32R, tag="h2",
                                  name=f"hq{q}") for q in range(4)]
                for q in range(4):
                    nc.sync.dma_start(hq[q][:], hT_r[:, 4 * q:4 * q + 4, sl])
                for sc in range(ST // 128):
                    ps = psAv.tile([128, NH * D], F32, tag="psv", name="psv")
                    for hc in range(HC):
                        nc.tensor.matmul(
                            ps[:],
                            hq[hc // 4][:, hc % 4, sc * 128:(sc + 1) * 128],
                            wv_sb[:, hc, :],
                            start=(hc == 0), stop=(hc == HC - 1),
                        )
                    nc.scalar.copy(
                        v_sb[:, st * (ST // 128) + sc, :], ps[:])

        # A^T (phase B -> C) reuses wv's slot; w_o prefetches during B
        at_all = wpool.tile([128, NH, S], F32R, tag="w", name="at")
        wop = ctx.enter_context(tc.tile_pool(name="cwo", bufs=1))
        wo_sb = wop.tile([128, NH, HID], F32R, tag="wo", name="wo")
        for g in range(NH):
            nc.sync.dma_start(wo_sb[:, g, :], woT_r[:, g, :])

        # ================= Phase B =================
        with ExitStack() as bctx:
            expp = bctx.enter_context(tc.tile_pool(name="bexp", bufs=6))
            smallp = bctx.enter_context(tc.tile_pool(name="bsmall", bufs=3))
            psS = bctx.enter_context(
                tc.tile_pool(name="bpss", bufs=2, space="PSUM"))
            psPV = bctx.enter_context(
                tc.tile_pool(name="bpspv", bufs=2, space="PSUM"))
            psCS = bctx.enter_context(
                tc.tile_pool(name="bpscs", bufs=2, space="PSUM"))

            for h in range(NH):
                for qt in range(NQT):
                    nallow = (QT // 128) * qt + (QT // 128)
                    qsl = bass.ts(qt, QT)
                    pvps = psPV.tile([128, QT], F32, tag="pv", name="pv")
                    csps = psCS.tile([1, QT], F32, tag="cs", name="cs")

                    # scores^T in 2-chunk PSUM groups; exp to SBUF groups
                    ngrp = (nallow + 1) // 2
                    egrp = []
                    for g in range(ngrp):
                        k0 = 2 * g
                        nk = min(2, nallow - k0)
                        sps = psS.tile([128, 2, QT], F32, tag="s", name="s")
                        eb = expp.tile([128, 2, QT], F32R, tag="e", name="e")
                        egrp.append(eb)
                        for i in range(nk):
                            kc = k0 + i
                            lo = max(0, 128 * (kc - 4 * qt))
                            nc.tensor.matmul(
                                sps[:, i, lo:QT],
                                ksb[:, h, kc * 128:(kc + 1) * 128],
                                qsb[:, h, qt * QT + lo:(qt + 1) * QT],
                                start=True, stop=True,
                            )
                        j0 = k0 - 4 * qt
                        if j0 + nk - 1 < 0:
                            nc.scalar.activation(
                                eb[:, 0:nk, :], sps[:, 0:nk, :],
                                mybir.ActivationFunctionType.Exp,
                                scale=DSCALE)
                        else:
                            for i in range(nk):
                                kc = k0 + i
                                j = kc - 4 * qt
                                lo = max(0, 128 * j)
                                nc.scalar.activation(
                                    eb[:, i, lo:QT], sps[:, i, lo:QT],
                                    mybir.ActivationFunctionType.Exp,
                                    scale=DSCALE)
                                if j >= 0:
                                    nc.vector.tensor_tensor(
                                        eb[:, i, lo:lo + 128],
                                        eb[:, i, lo:lo + 128].bitcast(F32),
                                        tri_sb[:],
                                        mybir.AluOpType.mult)

                    # colsum + PV accumulation over allowed chunks
                    for kc in range(nallow):
                        j = kc - 4 * qt
                        lo = max(0, 128 * j)
                        eb = egrp[kc // 2]
                        i = kc % 2
                        nc.tensor.matmul(
                            csps[:, lo:QT], ones_sb[:],
                            eb[:, i, lo:QT],
                            start=(kc == 0), stop=(kc == nallow - 1),
                            skip_group_check=True,
                        )
                        nc.tensor.matmul(
                            pvps[:, lo:QT],
                            v_sb[:, kc, h * D:(h + 1) * D],
                            eb[:, i, lo:QT],
                            start=(kc == 0), stop=(kc == nallow - 1),
                            skip_group_check=True,
                        )

                    # normalize: at = pv * broadcast(1/colsum). Broadcast
                    # via a K=1 PE matmul (ones column x reciprocal row).
                    rec = smallp.tile([1, QT], F32R, tag="rec", name="rec")
                    with nc.allow_low_precision(
                            reason="softmax denom reciprocal to f32r"):
                        nc.vector.reciprocal(rec[:], csps[:])
                    rbc = psPV.tile([128, QT], F32, tag="pv", name="rbc")
                    nc.tensor.matmul(rbc[:], onesr_sb[:], rec[:],
                                     start=True, stop=True)
                    at_t = smallp.tile([128, QT], F32, tag="att", name="att")
                    nc.vector.tensor_copy(at_t[:], pvps[:])
                    nc.vector.tensor_tensor(
                        at_all[:, h, qsl], at_t[:], rbc[:],
                        mybir.AluOpType.mult)

        # ================= Phase C =================
        with ExitStack() as cctx:
            outp = cctx.enter_context(tc.tile_pool(name="cout", bufs=3))
            psO = cctx.enter_context(
                tc.tile_pool(name="cpso", bufs=4, space="PSUM"))

            for sc in range(S // 128):
                ssl = bass.ts(sc, 128)
                ot = outp.tile([128, HID], F32, tag="ot", name="ot")
                for nt in range(HID // QT):
                    nsl = bass.ts(nt, QT)
                    ps = psO.tile([128, QT], F32, tag="o", name="o")
                    for g in range(NH):
                        nc.tensor.matmul(
                            ps[:],
                            at_all[:, g, ssl],
                            wo_sb[:, g, nsl],
                            start=(g == 0), stop=(g == NH - 1),
                        )
                    if nt % 2 == 0:
                        nc.vector.tensor_copy(ot[:, nsl], ps[:])
                    else:
                        nc.scalar.copy(ot[:, nsl], ps[:])
                nc.sync.dma_start(out.ap()[ssl, :], ot[:])

    nc.compile()
    return nc


def _prep_in_maps(hidden_states, cos, sin, w_qkv, w_o):
    hs = np.ascontiguousarray(np.asarray(hidden_states, dtype=np.float32))
    cos = np.asarray(cos, dtype=np.float32)
    sin = np.asarray(sin, dtype=np.float32)
    w_qkv = np.asarray(w_qkv, dtype=np.float32)
    w_o = np.asarray(w_o, dtype=np.float32)

    wT = np.ascontiguousarray(w_qkv.T)          # (HID, 3*H*D)
    woTf = np.ascontiguousarray(w_o.T)          # (H*D, HID)
    cosT = np.ascontiguousarray(cos.T)          # (D, S)
    sinT = np.ascontiguousarray(sin.T)
    sinS = sinT.copy()
    sinS[:64] = -sinT[:64]
    tri = np.triu(np.ones((128, 128), np.float32))
    ones = np.ones((128, 1), np.float32)

    hT = [np.ascontiguousarray(hs[b].T) for b in range(B)]

    in_maps = []
    for c in range(NCORES):
        b, hg = c // 4, c % 4
        lo, hi = hg * NH * D, (hg + 1) * NH * D
        in_maps.append({
            "hT": hT[b],
            "wqT": np.ascontiguousarray(wT[:, lo:hi]),
            "wkT": np.ascontiguousarray(wT[:, H * D + lo:H * D + hi]),
            "wvT": np.ascontiguousarray(wT[:, 2 * H * D + lo:2 * H * D + hi]),
            "woT": np.ascontiguousarray(woTf[lo:hi, :]),
            "cosT": cosT,
            "sinS": sinS,
            "tri": tri,
            "ones": ones,
            "onesr": np.ones((1, 128), np.float32),
        })
    return in_maps


def kernel(hidden_states, cos, sin, w_qkv, w_o, _trace=False):
    if "nc" not in _CACHED:
        _CACHED["nc"] = _build_nc()
    nc = _CACHED["nc"]
    in_maps = _prep_in_maps(hidden_states, cos, sin, w_qkv, w_o)
    res = run_bass_kernel_spmd(nc, in_maps, core_ids=list(range(NCORES)),
                               trace=_trace)
    _CACHED["last_result"] = res
    out = np.zeros((B, S, HID), np.float32)
    for c in range(NCORES):
        out[c // 4] += res.results[c]["out"]
    return out



# revision 2
# speedup vs baseline: 1.2020x; 1.2020x over previous
"""Trainium2 Bass kernel for AttentionWithRoPE (B=2, S=2048, HID=2048, H=16, D=128).

Sharding (8 cores): tensor-parallel over heads x data-parallel over batch.
Core c handles batch c//4 and heads 4*(c%4) .. 4*(c%4)+4.

Key structure (v2):
  - QKV projection runs as fp8e4m3 DoubleRow matmuls (2 k-chunks per
    instruction, 0.5 cyc/row) with 3-term error compensation: operands are
    split hi/lo on the HOST (x ~ x_hi + x_lo, both fp8) and the product is
    x_hi*w_hi + x_lo*w_hi + x_hi*w_lo (the lo*lo term is ~1e-3 relative and
    dropped). Weights are pre-scaled by 64 on the host so their magnitudes
    sit mid-range in fp8; the scale is undone for free in the RoPE cos/sin
    tables (Q,K) and in the PSUM->SBUF copy (V). Single fused pass over
    hidden produces Q (RoPE'd), K (RoPE'd) and V per s-tile, so hidden is
    loaded once (fp8 hi+lo = half the f32 bytes).
  - Attention per head in scores^T orientation ([k, q]); q/k/v/exp(p) all
    live in SBUF as fp16 (PE rate is the same, DVE gets 2-4x, SBUF/DMA
    halve). exp on ScalarE straight from PSUM with 1/sqrt(D) folded in;
    fully-masked k-blocks skipped; diagonal blocks masked with a 0/1
    upper-tri fp16 tile. The softmax denominator no longer burns PE
    columns per chunk: exp chunks are accumulated into an fp16 E_acc on
    DVE (scalar_tensor_tensor, 4x all-SBUF fp16 mode) and ONE ones-vector
    PE matmul per (head, q-tile) reduces E_acc's 128 partitions.
    Normalization: reciprocal + K=1 PE-matmul broadcast + DVE multiply.
  - Output projection with the core's fp16 w_o column slice; the host sums
    the four partials per batch (the TP reduce).
  - DMA issue order is arranged so the first Q matmul only waits for
    wq_hi + the first hidden chunk (~1.5 MB), not the whole weight set.
"""
import numpy as np
import ml_dtypes
from contextlib import ExitStack

import concourse.bass as bass
import concourse.tile as tile
from concourse import bacc, mybir
from concourse.bass_utils import run_bass_kernel_spmd

B, S, HID = 2, 2048, 2048
H, D = 16, 128
NCORES = 8
NH = 4                 # heads per core
HC = HID // 128        # hid chunks
HCP = HC // 2          # hid chunk pairs (DoubleRow)
AST = 512              # phase-A s-tile width
ANST = S // AST
QT = 512               # phase-B q-tile width
NQT = S // QT
DSCALE = float(D) ** -0.5
ALPHA = 64.0           # host-side weight scale for fp8
F32 = mybir.dt.float32
F32R = mybir.dt.float32r
F16 = mybir.dt.float16
F8 = mybir.dt.float8e4
DR = mybir.MatmulPerfMode.DoubleRow

_CACHED = {}


def _build_nc():
    nc = bacc.Bacc("TRN2", target_bir_lowering=False, debug=False,
                   num_devices=NCORES)
    h_hi = nc.dram_tensor("h_hi", [HID, S], F8, kind="ExternalInput")
    h_lo = nc.dram_tensor("h_lo", [HID, S], F8, kind="ExternalInput")
    wqh = nc.dram_tensor("wqh", [HID, NH * D], F8, kind="ExternalInput")
    wql = nc.dram_tensor("wql", [HID, NH * D], F8, kind="ExternalInput")
    wkh = nc.dram_tensor("wkh", [HID, NH * D], F8, kind="ExternalInput")
    wkl = nc.dram_tensor("wkl", [HID, NH * D], F8, kind="ExternalInput")
    wvh = nc.dram_tensor("wvh", [HID, NH * D], F8, kind="ExternalInput")
    wvl = nc.dram_tensor("wvl", [HID, NH * D], F8, kind="ExternalInput")
    woT = nc.dram_tensor("woT", [NH * D, HID], F16, kind="ExternalInput")
    cosT = nc.dram_tensor("cosT", [D, S], F32, kind="ExternalInput")
    sinS = nc.dram_tensor("sinS", [D, S], F32, kind="ExternalInput")
    tri = nc.dram_tensor("tri", [128, 128], F16, kind="ExternalInput")
    ones = nc.dram_tensor("ones", [128, 1], F16, kind="ExternalInput")
    onesr = nc.dram_tensor("onesr", [1, 128], F32R, kind="ExternalInput")
    out = nc.dram_tensor("out", [S, HID], F32, kind="ExternalOutput")

    hh_r = h_hi.ap().rearrange("(hc p) s -> p hc s", p=128)
    hl_r = h_lo.ap().rearrange("(hc p) s -> p hc s", p=128)
    w_r = {w.name: w.ap().rearrange("(hc p) m -> p hc m", p=128)
           for w in (wqh, wql, wkh, wkl, wvh, wvl)}
    woT_r = woT.ap().rearrange("(g p) n -> p g n", p=128)

    with tile.TileContext(nc) as tc, ExitStack() as ctx:
        # ---- small constants ----
        constp = ctx.enter_context(tc.tile_pool(name="const", bufs=1))
        tri_sb = constp.tile([128, 128], F16, tag="tri", name="tri")
        nc.sync.dma_start(tri_sb[:], tri.ap())
        ones_sb = constp.tile([128, 1], F16, tag="ones", name="ones")
        nc.sync.dma_start(ones_sb[:], ones.ap())
        onesr_sb = constp.tile([1, 128], F32R, tag="onesr", name="onesr")
        nc.sync.dma_start(onesr_sb[:], onesr.ap())

        # Q^T/K^T stay resident in SBUF through attention (fp16)
        qkp = ctx.enter_context(tc.tile_pool(name="qk", bufs=1))
        qsb = qkp.tile([128, NH, S], F16, tag="qsb", name="qsb")
        ksb = qkp.tile([128, NH, S], F16, tag="ksb", name="ksb")
        vp = ctx.enter_context(tc.tile_pool(name="vp", bufs=1))
        v_sb = vp.tile([128, S // 128, NH * D], F16, tag="vsb", name="vsb")

        # fp8 hi/lo weights, all resident
        wp = ctx.enter_context(tc.tile_pool(name="w", bufs=1))
        wsb = {name: wp.tile([128, HC, NH * D], F8, tag=name, name=name)
               for name in ("wqh", "wql", "wkh", "wkl", "wvh", "wvl")}

        # ================= Phase A: fused QKV =================
        with ExitStack() as astack:
            hpool = astack.enter_context(tc.tile_pool(name="ah", bufs=3))
            cspool = astack.enter_context(tc.tile_pool(name="acs", bufs=2))
            ropep = astack.enter_context(tc.tile_pool(name="arope", bufs=1))
            psA = astack.enter_context(
                tc.tile_pool(name="apsqk", bufs=5, space="PSUM"))
            psV = astack.enter_context(
                tc.tile_pool(name="apsv", bufs=3, space="PSUM"))

            def load_tile(st):
                sl = bass.ts(st, AST)
                hh = hpool.tile([128, HC, AST], F8, tag="hh", name="hh")
                hl = hpool.tile([128, HC, AST], F8, tag="hl", name="hl")
                cs_t = cspool.tile([128, AST], F32, tag="cs", name="cs")
                ss_t = cspool.tile([128, AST], F32, tag="ss", name="ss")
                if st == 0:
                    # ordered so the first matmuls' inputs land first
                    nc.sync.dma_start(wsb["wqh"][:], w_r["wqh"])
                    for c in range(4):
                        nc.sync.dma_start(hh[:, 4 * c:4 * c + 4, :],
                                          hh_r[:, 4 * c:4 * c + 4, sl])
                    nc.sync.dma_start(wsb["wql"][:], w_r["wql"])
                    nc.sync.dma_start(cs_t[:], cosT.ap()[:, sl])
                    nc.sync.dma_start(ss_t[:], sinS.ap()[:, sl])
                    for c in range(4):
                        nc.sync.dma_start(hl[:, 4 * c:4 * c + 4, :],
                                          hl_r[:, 4 * c:4 * c + 4, sl])
                    for name in ("wkh", "wkl", "wvh", "wvl"):
                        nc.sync.dma_start(wsb[name][:], w_r[name])
                else:
                    for c in range(4):
                        nc.sync.dma_start(hh[:, 4 * c:4 * c + 4, :],
                                          hh_r[:, 4 * c:4 * c + 4, sl])
                    nc.sync.dma_start(cs_t[:], cosT.ap()[:, sl])
                    nc.sync.dma_start(ss_t[:], sinS.ap()[:, sl])
                    for c in range(4):
                        nc.sync.dma_start(hl[:, 4 * c:4 * c + 4, :],
                                          hl_r[:, 4 * c:4 * c + 4, sl])
                return hh, hl, cs_t, ss_t

            for st in range(ANST):
                sl = bass.ts(st, AST)
                hh, hl, cs_t, ss_t = load_tile(st)

                for wn, dsb in (("wq", qsb), ("wk", ksb)):
                    whi, wlo = wsb[wn + "h"], wsb[wn + "l"]
                    for h in range(NH):
                        hsl = slice(h * D, (h + 1) * D)
                        ps = psA.tile([128, AST], F32, tag="psqk",
                                      name="psqk")
                        k = 0
                        for wt, ht in ((whi, hh), (wlo, hh), (whi, hl)):
                            for p in range(HCP):
                                nc.tensor.matmul(
                                    ps[:],
                                    wt[:, 2 * p:2 * p + 2, hsl],
                                    ht[:, 2 * p:2 * p + 2, :],
                                    start=(k == 0), stop=(k == 3 * HCP - 1),
                                    perf_mode=DR,
                                )
                                k += 1
                        # RoPE: out = x*cos + shift(x)*sin_signed, with the
                        # 1/ALPHA weight descale folded into the host tables.
                        tsin = ropep.tile([128, AST], F32, tag="tsin",
                                          name="tsin")
                        nc.vector.tensor_tensor(
                            tsin[0:64, :], ps[64:128, :], ss_t[0:64, :],
                            mybir.AluOpType.mult)
                        nc.vector.tensor_tensor(
                            tsin[64:128, :], ps[0:64, :], ss_t[64:128, :],
                            mybir.AluOpType.mult)
                        tcos = ropep.tile([128, AST], F32, tag="tcos",
                                          name="tcos")
                        nc.vector.tensor_tensor(
                            tcos[:], ps[:], cs_t[:], mybir.AluOpType.mult)
                        nc.vector.scalar_tensor_tensor(
                            dsb[:, h, sl], tcos[:], 1.0, tsin[:],
                            mybir.AluOpType.mult, mybir.AluOpType.add)

                # V: natural orientation, 1/ALPHA descale in the PSUM copy
                for sc in range(AST // 128):
                    scl = slice(sc * 128, (sc + 1) * 128)
                    ps = psV.tile([128, NH * D], F32, tag="psv", name="psv")
                    k = 0
                    for wt, ht in ((wsb["wvh"], hh), (wsb["wvl"], hh),
                                   (wsb["wvh"], hl)):
                        for p in range(HCP):
                            nc.tensor.matmul(
                                ps[:],
                                ht[:, 2 * p:2 * p + 2, scl],
                                wt[:, 2 * p:2 * p + 2, :],
                                start=(k == 0), stop=(k == 3 * HCP - 1),
                                perf_mode=DR,
                            )
                            k += 1
                    nc.scalar.mul(
                        v_sb[:, st * (AST // 128) + sc, :], ps[:],
                        1.0 / ALPHA)

        # w_o prefetches during phase B
        wop = ctx.enter_context(tc.tile_pool(name="cwo", bufs=1))
        wo_sb = wop.tile([128, NH, HID], F16, tag="wo", name="wo")
        for g in range(NH):
            nc.sync.dma_start(wo_sb[:, g, :], woT_r[:, g, :])
        atp = ctx.enter_context(tc.tile_pool(name="at", bufs=1))
        at_all = atp.tile([128, NH, S], F16, tag="at", name="at")

        # ================= Phase B =================
        with ExitStack() as bctx:
            expp = bctx.enter_context(tc.tile_pool(name="bexp", bufs=8))
            eaccp = bctx.enter_context(tc.tile_pool(name="beacc", bufs=2))
            smallp = bctx.enter_context(tc.tile_pool(name="bsmall", bufs=3))
            psS = bctx.enter_context(
                tc.tile_pool(name="bpss", bufs=2, space="PSUM"))
            psPV = bctx.enter_context(
                tc.tile_pool(name="bpspv", bufs=2, space="PSUM"))
            psCS = bctx.enter_context(
                tc.tile_pool(name="bpscs", bufs=1, space="PSUM"))
            psRB = bctx.enter_context(
                tc.tile_pool(name="bpsrb", bufs=1, space="PSUM"))

            for h in range(NH):
                for qt in range(NQT):
                    nallow = (QT // 128) * qt + (QT // 128)
                    qsl = bass.ts(qt, QT)
                    pvps = psPV.tile([128, QT], F32, tag="pv", name="pv")
                    csps = psCS.tile([1, QT], F32, tag="cs", name="cs")
                    eacc = eaccp.tile([128, QT], F16, tag="ea", name="ea")

                    # scores^T in 2-chunk PSUM groups; exp to fp16 SBUF;
                    # DVE accumulates exp chunks into eacc (all-SBUF fp16
                    # scalar_tensor_tensor runs at 4x).
                    ngrp = (nallow + 1) // 2
                    egrp = []
                    for g in range(ngrp):
                        k0 = 2 * g
                        nk = min(2, nallow - k0)
                        sps = psS.tile([128, 2, QT], F32, tag="s", name="s")
                        eb = expp.tile([128, 2, QT], F16, tag="e", name="e")
                        egrp.append(eb)
                        for i in range(nk):
                            kc = k0 + i
                            lo = max(0, 128 * (kc - 4 * qt))
                            nc.tensor.matmul(
                                sps[:, i, lo:QT],
                                ksb[:, h, kc * 128:(kc + 1) * 128],
                                qsb[:, h, qt * QT + lo:(qt + 1) * QT],
                                start=True, stop=True,
                            )
                        j0 = k0 - 4 * qt
                        if j0 + nk - 1 < 0:
                            nc.scalar.activation(
                                eb[:, 0:nk, :], sps[:, 0:nk, :],
                                mybir.ActivationFunctionType.Exp,
                                scale=DSCALE)
                        else:
                            for i in range(nk):
                                kc = k0 + i
                                j = kc - 4 * qt
                                lo = max(0, 128 * j)
                                nc.scalar.activation(
                                    eb[:, i, lo:QT], sps[:, i, lo:QT],
                                    mybir.ActivationFunctionType.Exp,
                                    scale=DSCALE)
                                if j >= 0:
                                    nc.vector.tensor_tensor(
                                        eb[:, i, lo:lo + 128],
                                        eb[:, i, lo:lo + 128],
                                        tri_sb[:],
                                        mybir.AluOpType.mult)
                        for i in range(nk):
                            kc = k0 + i
                            lo = max(0, 128 * (kc - 4 * qt))
                            if kc == 0:
                                nc.vector.tensor_copy(
                                    eacc[:, lo:QT], eb[:, i, lo:QT])
                            else:
                                nc.vector.scalar_tensor_tensor(
                                    eacc[:, lo:QT], eb[:, i, lo:QT], 1.0,
                                    eacc[:, lo:QT],
                                    mybir.AluOpType.mult,
                                    mybir.AluOpType.add)

                    # PV accumulation over allowed chunks
                    for kc in range(nallow):
                        j = kc - 4 * qt
                        lo = max(0, 128 * j)
                        eb = egrp[kc // 2]
                        i = kc % 2
                        nc.tensor.matmul(
                            pvps[:, lo:QT],
                            v_sb[:, kc, h * D:(h + 1) * D],
                            eb[:, i, lo:QT],
                            start=(kc == 0), stop=(kc == nallow - 1),
                            skip_group_check=True,
                        )

                    # single colsum matmul over the accumulated exp
                    nc.tensor.matmul(csps[:], ones_sb[:], eacc[:],
                                     start=True, stop=True)

                    # normalize: at = pv * broadcast(1/colsum). Broadcast
                    # via a K=1 PE matmul (ones column x reciprocal row).
                    rec = smallp.tile([1, QT], F32R, tag="rec", name="rec")
                    with nc.allow_low_precision(
                            reason="softmax denom reciprocal to f32r"):
                        nc.vector.reciprocal(rec[:], csps[:])
                    rbc = psRB.tile([128, QT], F32, tag="rb", name="rb")
                    nc.tensor.matmul(rbc[:], onesr_sb[:], rec[:],
                                     start=True, stop=True)
                    at_t = smallp.tile([128, QT], F32, tag="att", name="att")
                    nc.vector.tensor_copy(at_t[:], pvps[:])
                    nc.vector.tensor_tensor(
                        at_all[:, h, qsl], at_t[:], rbc[:],
                        mybir.AluOpType.mult)

        # ================= Phase C =================
        with ExitStack() as cctx:
            outp = cctx.enter_context(tc.tile_pool(name="cout", bufs=4))
            psO = cctx.enter_context(
                tc.tile_pool(name="cpso", bufs=4, space="PSUM"))

            for sc in range(S // 128):
                ssl = bass.ts(sc, 128)
                for nt in range(HID // QT):
                    nsl = bass.ts(nt, QT)
                    ps = psO.tile([128, QT], F32, tag="o", name="o")
                    for g in range(NH):
                        nc.tensor.matmul(
                            ps[:],
                            at_all[:, g, ssl],
                            wo_sb[:, g, nsl],
                            start=(g == 0), stop=(g == NH - 1),
                        )
                    ot = outp.tile([128, QT], F32, tag="ot", name="ot")
                    if nt % 2 == 0:
                        nc.vector.tensor_copy(ot[:], ps[:])
                    else:
                        nc.scalar.copy(ot[:], ps[:])
                    nc.sync.dma_start(out.ap()[ssl, nsl], ot[:])

    nc.compile()
    return nc


def _fp8_split(x):
    hi = x.astype(ml_dtypes.float8_e4m3)
    lo = (x - hi.astype(np.float32)).astype(ml_dtypes.float8_e4m3)
    return (np.ascontiguousarray(hi).view(np.uint8),
            np.ascontiguousarray(lo).view(np.uint8))


def _prep_in_maps(hidden_states, cos, sin, w_qkv, w_o):
    hs = np.ascontiguousarray(np.asarray(hidden_states, dtype=np.float32))
    cos = np.asarray(cos, dtype=np.float32)
    sin = np.asarray(sin, dtype=np.float32)
    w_qkv = np.asarray(w_qkv, dtype=np.float32)
    w_o = np.asarray(w_o, dtype=np.float32)

    wT = np.ascontiguousarray(w_qkv.T) * ALPHA   # (HID, 3*H*D), pre-scaled
    woTf = np.ascontiguousarray(w_o.T)           # (H*D, HID)
    cosT = np.ascontiguousarray(cos.T) / ALPHA   # descale folded in
    sinT = np.ascontiguousarray(sin.T)
    sinS = sinT.copy()
    sinS[:64] = -sinT[:64]
    sinS /= ALPHA
    tri = np.triu(np.ones((128, 128), np.float16))
    ones = np.ones((128, 1), np.float16)
    onesr = np.ones((1, 128), np.float32)

    h_split = [_fp8_split(np.ascontiguousarray(hs[b].T)) for b in range(B)]
    w_split = []                                 # per head-group hi/lo
    for hg in range(4):
        lo_, hi_ = hg * NH * D, (hg + 1) * NH * D
        w_split.append({
            "wq": _fp8_split(np.ascontiguousarray(wT[:, lo_:hi_])),
            "wk": _fp8_split(np.ascontiguousarray(
                wT[:, H * D + lo_:H * D + hi_])),
            "wv": _fp8_split(np.ascontiguousarray(
                wT[:, 2 * H * D + lo_:2 * H * D + hi_])),
            "wo": np.ascontiguousarray(woTf[lo_:hi_, :]).astype(np.float16),
        })

    in_maps = []
    for c in range(NCORES):
        b, hg = c // 4, c % 4
        ws = w_split[hg]
        in_maps.append({
            "h_hi": h_split[b][0],
            "h_lo": h_split[b][1],
            "wqh": ws["wq"][0], "wql": ws["wq"][1],
            "wkh": ws["wk"][0], "wkl": ws["wk"][1],
            "wvh": ws["wv"][0], "wvl": ws["wv"][1],
            "woT": ws["wo"],
            "cosT": cosT,
            "sinS": sinS,
            "tri": tri,
            "ones": ones,
            "onesr": onesr,
        })
    return in_maps


def kernel(hidden_states, cos, sin, w_qkv, w_o, _trace=False):
    if "nc" not in _CACHED:
        _CACHED["nc"] = _build_nc()
    nc = _CACHED["nc"]
    in_maps = _prep_in_maps(hidden_states, cos, sin, w_qkv, w_o)
    res = run_bass_kernel_spmd(nc, in_maps, core_ids=list(range(NCORES)),
                               trace=_trace)
    _CACHED["last_result"] = res
    out = np.zeros((B, S, HID), np.float32)
    for c in range(NCORES):
        out[c // 4] += res.results[c]["out"]
    return out


# revision 15
# speedup vs baseline: 1.2589x; 1.0473x over previous
"""Trainium2 Bass kernel for AttentionWithRoPE (B=2, S=2048, HID=2048, H=16, D=128).

Sharding (8 cores): tensor-parallel over heads x data-parallel over batch.
Core c handles batch c//4 and heads 4*(c%4) .. 4*(c%4)+4.

Key structure (v2):
  - QKV projection runs as fp8e4m3 DoubleRow matmuls (2 k-chunks per
    instruction, 0.5 cyc/row) with 3-term error compensation: operands are
    split hi/lo on the HOST (x ~ x_hi + x_lo, both fp8) and the product is
    x_hi*w_hi + x_lo*w_hi + x_hi*w_lo (the lo*lo term is ~1e-3 relative and
    dropped). Weights are pre-scaled by 64 on the host so their magnitudes
    sit mid-range in fp8; the scale is undone for free in the RoPE cos/sin
    tables (Q,K) and in the PSUM->SBUF copy (V). Single fused pass over
    hidden produces Q (RoPE'd), K (RoPE'd) and V per s-tile, so hidden is
    loaded once (fp8 hi+lo = half the f32 bytes).
  - Attention per head in scores^T orientation ([k, q]); q/k/v/exp(p) all
    live in SBUF as fp16 (PE rate is the same, DVE gets 2-4x, SBUF/DMA
    halve). exp on ScalarE straight from PSUM with 1/sqrt(D) folded in;
    fully-masked k-blocks skipped; diagonal blocks masked with a 0/1
    upper-tri fp16 tile. The softmax denominator no longer burns PE
    columns per chunk: exp chunks are accumulated into an fp16 E_acc on
    DVE (scalar_tensor_tensor, 4x all-SBUF fp16 mode) and ONE ones-vector
    PE matmul per (head, q-tile) reduces E_acc's 128 partitions.
    Normalization: reciprocal + K=1 PE-matmul broadcast + DVE multiply.
  - Output projection with the core's fp16 w_o column slice; the host sums
    the four partials per batch (the TP reduce).
  - DMA issue order is arranged so the first Q matmul only waits for
    wq_hi + the first hidden chunk (~1.5 MB), not the whole weight set.
"""
import numpy as np
import ml_dtypes
from contextlib import ExitStack

import concourse.bass as bass
import concourse.tile as tile
from concourse import bacc, mybir
from concourse.bass_utils import run_bass_kernel_spmd

B, S, HID = 2, 2048, 2048
H, D = 16, 128
NCORES = 8
NH = 4                 # heads per core
HC = HID // 128        # hid chunks
HCP = HC // 2          # hid chunk pairs (DoubleRow)
AST = 512              # phase-A s-tile width
ANST = S // AST
QT = 512               # phase-B q-tile width
NQT = S // QT
DSCALE = float(D) ** -0.5
ALPHA = 64.0           # host-side weight scale for fp8
F32 = mybir.dt.float32
F32R = mybir.dt.float32r
F16 = mybir.dt.float16
F8 = mybir.dt.float8e4
DR = mybir.MatmulPerfMode.DoubleRow

_CACHED = {}


def _build_nc():
    nc = bacc.Bacc("TRN2", target_bir_lowering=False, debug=False,
                   num_devices=NCORES)
    h_hi = nc.dram_tensor("h_hi", [HID, S], F8, kind="ExternalInput")
    h_lo = nc.dram_tensor("h_lo", [HID, S], F8, kind="ExternalInput")
    wqh = nc.dram_tensor("wqh", [HID, NH * D], F8, kind="ExternalInput")
    wql = nc.dram_tensor("wql", [HID, NH * D], F8, kind="ExternalInput")
    wkh = nc.dram_tensor("wkh", [HID, NH * D], F8, kind="ExternalInput")
    wkl = nc.dram_tensor("wkl", [HID, NH * D], F8, kind="ExternalInput")
    wvh = nc.dram_tensor("wvh", [HID, NH * D], F8, kind="ExternalInput")
    wvl = nc.dram_tensor("wvl", [HID, NH * D], F8, kind="ExternalInput")
    woT = nc.dram_tensor("woT", [NH * D, HID], F16, kind="ExternalInput")
    cosT = nc.dram_tensor("cosT", [D, S], F16, kind="ExternalInput")
    sinS = nc.dram_tensor("sinS", [D, S], F16, kind="ExternalInput")
    tri = nc.dram_tensor("tri", [128, 128], F16, kind="ExternalInput")
    ones = nc.dram_tensor("ones", [128, 1], F16, kind="ExternalInput")
    onesr = nc.dram_tensor("onesr", [1, 128], F32R, kind="ExternalInput")
    out = nc.dram_tensor("out", [S, HID], F16, kind="ExternalOutput")

    hh_r = h_hi.ap().rearrange("(hc p) s -> p hc s", p=128)
    hl_r = h_lo.ap().rearrange("(hc p) s -> p hc s", p=128)
    w_r = {w.name: w.ap().rearrange("(hc p) m -> p hc m", p=128)
           for w in (wqh, wql, wkh, wkl, wvh, wvl)}
    woT_r = woT.ap().rearrange("(g p) n -> p g n", p=128)

    with tile.TileContext(nc) as tc, ExitStack() as ctx:
        # ---- small constants ----
        constp = ctx.enter_context(tc.tile_pool(name="const", bufs=1))
        tri_sb = constp.tile([128, 128], F16, tag="tri", name="tri")
        nc.sync.dma_start(tri_sb[:], tri.ap())
        ones_sb = constp.tile([128, 1], F16, tag="ones", name="ones")
        nc.sync.dma_start(ones_sb[:], ones.ap())
        onesr_sb = constp.tile([1, 128], F32R, tag="onesr", name="onesr")
        nc.sync.dma_start(onesr_sb[:], onesr.ap())

        # Q^T/K^T stay resident in SBUF through attention (fp16)
        qkp = ctx.enter_context(tc.tile_pool(name="qk", bufs=1))
        qsb = qkp.tile([128, NH, S], F16, tag="qsb", name="qsb")
        ksb = qkp.tile([128, NH, S], F16, tag="ksb", name="ksb")
        vp = ctx.enter_context(tc.tile_pool(name="vp", bufs=1))
        v_sb = vp.tile([128, S // 128, NH * D], F16, tag="vsb", name="vsb")

        # fp8 hi/lo weights, all resident
        wp = ctx.enter_context(tc.tile_pool(name="w", bufs=1))
        wsb = {name: wp.tile([128, HC, NH * D], F8, tag=name, name=name)
               for name in ("wqh", "wql", "wkh", "wkl", "wvh", "wvl")}

        # ================= Phase A: fused QKV =================
        with ExitStack() as astack:
            hpool = astack.enter_context(tc.tile_pool(name="ah", bufs=3))
            cspool = astack.enter_context(tc.tile_pool(name="acs", bufs=2))
            ropep = astack.enter_context(tc.tile_pool(name="arope", bufs=1))
            psA = astack.enter_context(
                tc.tile_pool(name="apsqk", bufs=5, space="PSUM"))
            psV = astack.enter_context(
                tc.tile_pool(name="apsv", bufs=3, space="PSUM"))

            def load_tile(st):
                sl = bass.ts(st, AST)
                hh = hpool.tile([128, HC, AST], F8, tag="hh", name="hh")
                hl = hpool.tile([128, HC, AST], F8, tag="hl", name="hl")
                cs_t = cspool.tile([128, AST], F16, tag="cs", name="cs")
                ss_t = cspool.tile([128, AST], F16, tag="ss", name="ss")
                if st == 0:
                    # ordered so the first matmuls' inputs land first
                    nc.sync.dma_start(wsb["wqh"][:], w_r["wqh"])
                    for c in range(4):
                        nc.sync.dma_start(hh[:, 4 * c:4 * c + 4, :],
                                          hh_r[:, 4 * c:4 * c + 4, sl])
                    nc.sync.dma_start(wsb["wql"][:], w_r["wql"])
                    nc.sync.dma_start(cs_t[:], cosT.ap()[:, sl])
                    nc.sync.dma_start(ss_t[:], sinS.ap()[:, sl])
                    for c in range(4):
                        nc.sync.dma_start(hl[:, 4 * c:4 * c + 4, :],
                                          hl_r[:, 4 * c:4 * c + 4, sl])
                    for name in ("wkh", "wkl", "wvh", "wvl"):
                        nc.sync.dma_start(wsb[name][:], w_r[name])
                else:
                    for c in range(4):
                        nc.sync.dma_start(hh[:, 4 * c:4 * c + 4, :],
                                          hh_r[:, 4 * c:4 * c + 4, sl])
                    nc.sync.dma_start(cs_t[:], cosT.ap()[:, sl])
                    nc.sync.dma_start(ss_t[:], sinS.ap()[:, sl])
                    for c in range(4):
                        nc.sync.dma_start(hl[:, 4 * c:4 * c + 4, :],
                                          hl_r[:, 4 * c:4 * c + 4, sl])
                return hh, hl, cs_t, ss_t

            for st in range(ANST):
                sl = bass.ts(st, AST)
                hh, hl, cs_t, ss_t = load_tile(st)

                for wn, dsb in (("wq", qsb), ("wk", ksb)):
                    whi, wlo = wsb[wn + "h"], wsb[wn + "l"]
                    for h in range(NH):
                        hsl = slice(h * D, (h + 1) * D)
                        ps = psA.tile([128, AST], F32, tag="psqk",
                                      name="psqk")
                        k = 0
                        for wt, ht in ((whi, hh), (wlo, hh), (whi, hl)):
                            for p in range(HCP):
                                nc.tensor.matmul(
                                    ps[:],
                                    wt[:, 2 * p:2 * p + 2, hsl],
                                    ht[:, 2 * p:2 * p + 2, :],
                                    start=(k == 0), stop=(k == 3 * HCP - 1),
                                    perf_mode=DR,
                                )
                                k += 1
                        # RoPE: out = x*cos + shift(x)*sin_signed, with the
                        # 1/ALPHA weight descale folded into the host tables.
                        # PSUM-reading muls must stay on DVE (GPSIMD cannot
                        # access PSUM); the all-SBUF fp16 add runs on the
                        # idle Pool engine.
                        tsin = ropep.tile([128, AST], F16, tag="tsin",
                                          name="tsin")
                        nc.vector.tensor_tensor(
                            tsin[0:64, :], ps[64:128, :], ss_t[0:64, :],
                            mybir.AluOpType.mult)
                        nc.vector.tensor_tensor(
                            tsin[64:128, :], ps[0:64, :], ss_t[64:128, :],
                            mybir.AluOpType.mult)
                        tcos = ropep.tile([128, AST], F16, tag="tcos",
                                          name="tcos")
                        nc.vector.tensor_tensor(
                            tcos[:], ps[:], cs_t[:], mybir.AluOpType.mult)
                        nc.gpsimd.tensor_tensor(
                            dsb[:, h, sl], tcos[:], tsin[:],
                            mybir.AluOpType.add)

                # V: natural orientation, 1/ALPHA descale in the PSUM copy
                for sc in range(AST // 128):
                    scl = slice(sc * 128, (sc + 1) * 128)
                    ps = psV.tile([128, NH * D], F32, tag="psv", name="psv")
                    k = 0
                    for wt, ht in ((wsb["wvh"], hh), (wsb["wvl"], hh),
                                   (wsb["wvh"], hl)):
                        for p in range(HCP):
                            nc.tensor.matmul(
                                ps[:],
                                ht[:, 2 * p:2 * p + 2, scl],
                                wt[:, 2 * p:2 * p + 2, :],
                                start=(k == 0), stop=(k == 3 * HCP - 1),
                                perf_mode=DR,
                            )
                            k += 1
                    nc.scalar.mul(
                        v_sb[:, st * (AST // 128) + sc, :], ps[:],
                        1.0 / ALPHA)

        # w_o prefetches during phase B
        wop = ctx.enter_context(tc.tile_pool(name="cwo", bufs=1))
        wo_sb = wop.tile([128, NH, HID], F16, tag="wo", name="wo")
        for g in range(NH):
            nc.sync.dma_start(wo_sb[:, g, :], woT_r[:, g, :])
        atp = ctx.enter_context(tc.tile_pool(name="at", bufs=1))
        at_all = atp.tile([128, NH, S], F16, tag="at", name="at")

        # ================= Phase B =================
        with ExitStack() as bctx:
            expp = bctx.enter_context(tc.tile_pool(name="bexp", bufs=8))
            eaccp = bctx.enter_context(tc.tile_pool(name="beacc", bufs=2))
            smallp = bctx.enter_context(tc.tile_pool(name="bsmall", bufs=3))
            psS = bctx.enter_context(
                tc.tile_pool(name="bpss", bufs=2, space="PSUM"))
            psPV = bctx.enter_context(
                tc.tile_pool(name="bpspv", bufs=2, space="PSUM"))
            psCS = bctx.enter_context(
                tc.tile_pool(name="bpscs", bufs=1, space="PSUM"))
            psRB = bctx.enter_context(
                tc.tile_pool(name="bpsrb", bufs=1, space="PSUM"))

            for h in range(NH):
                for qt in range(NQT):
                    nallow = (QT // 128) * qt + (QT // 128)
                    qsl = bass.ts(qt, QT)
                    pvps = psPV.tile([128, QT], F32, tag="pv", name="pv")
                    csps = psCS.tile([1, QT], F32, tag="cs", name="cs")
                    # split exp accumulators: DVE takes off-diagonal chunks,
                    # Pool takes diagonal ones (both engines run in parallel;
                    # diagonal chunks start at lo=0 so epool inits full-width)
                    edve = eaccp.tile([128, QT], F16, tag="ea", name="ea")
                    epool = eaccp.tile([128, QT], F16, tag="ep", name="ep")
                    dve_init = pool_init = False

                    # scores^T in 2-chunk PSUM groups; exp to fp16 SBUF;
                    # DVE accumulates exp chunks into eacc (all-SBUF fp16
                    # scalar_tensor_tensor runs at 4x).
                    ngrp = (nallow + 1) // 2
                    egrp = []
                    for g in range(ngrp):
                        k0 = 2 * g
                        nk = min(2, nallow - k0)
                        sps = psS.tile([128, 2, QT], F32, tag="s", name="s")
                        eb = expp.tile([128, 2, QT], F16, tag="e", name="e")
                        egrp.append(eb)
                        for i in range(nk):
                            kc = k0 + i
                            lo = max(0, 128 * (kc - 4 * qt))
                            nc.tensor.matmul(
                                sps[:, i, lo:QT],
                                ksb[:, h, kc * 128:(kc + 1) * 128],
                                qsb[:, h, qt * QT + lo:(qt + 1) * QT],
                                start=True, stop=True,
                            )
                        j0 = k0 - 4 * qt
                        if j0 + nk - 1 < 0:
                            nc.scalar.activation(
                                eb[:, 0:nk, :], sps[:, 0:nk, :],
                                mybir.ActivationFunctionType.Exp,
                                scale=DSCALE)
                        else:
                            for i in range(nk):
                                kc = k0 + i
                                j = kc - 4 * qt
                                lo = max(0, 128 * j)
                                nc.scalar.activation(
                                    eb[:, i, lo:QT], sps[:, i, lo:QT],
                                    mybir.ActivationFunctionType.Exp,
                                    scale=DSCALE)
                                if j >= 0:
                                    nc.vector.tensor_tensor(
                                        eb[:, i, lo:lo + 128],
                                        eb[:, i, lo:lo + 128],
                                        tri_sb[:],
                                        mybir.AluOpType.mult)
                        for i in range(nk):
                            kc = k0 + i
                            lo = max(0, 128 * (kc - 4 * qt))
                            if kc - 4 * qt >= 0:      # diagonal: Pool engine
                                if not pool_init:
                                    nc.gpsimd.tensor_copy(
                                        epool[:, lo:QT], eb[:, i, lo:QT])
                                    pool_init = True
                                else:
                                    nc.gpsimd.tensor_tensor(
                                        epool[:, lo:QT], eb[:, i, lo:QT],
                                        epool[:, lo:QT], mybir.AluOpType.add)
                            else:
                                if not dve_init:
                                    nc.vector.tensor_copy(
                                        edve[:, lo:QT], eb[:, i, lo:QT])
                                    dve_init = True
                                else:
                                    nc.vector.tensor_tensor(
                                        edve[:, lo:QT], eb[:, i, lo:QT],
                                        edve[:, lo:QT], mybir.AluOpType.add)

                    # PV accumulation over allowed chunks
                    for kc in range(nallow):
                        j = kc - 4 * qt
                        lo = max(0, 128 * j)
                        eb = egrp[kc // 2]
                        i = kc % 2
                        nc.tensor.matmul(
                            pvps[:, lo:QT],
                            v_sb[:, kc, h * D:(h + 1) * D],
                            eb[:, i, lo:QT],
                            start=(kc == 0), stop=(kc == nallow - 1),
                            skip_group_check=True,
                        )

                    # colsum over the (up to two) exp accumulators
                    accs = ([edve] if dve_init else []) + [epool]
                    for ai, acc in enumerate(accs):
                        nc.tensor.matmul(csps[:], ones_sb[:], acc[:],
                                         start=(ai == 0),
                                         stop=(ai == len(accs) - 1),
                                         skip_group_check=True)

                    # normalize: at = pv * broadcast(1/colsum). Broadcast
                    # via a K=1 PE matmul (ones column x reciprocal row).
                    rec = smallp.tile([1, QT], F32R, tag="rec", name="rec")
                    with nc.allow_low_precision(
                            reason="softmax denom reciprocal to f32r"):
                        nc.vector.reciprocal(rec[:], csps[:])
                    rbc = psRB.tile([128, QT], F32, tag="rb", name="rb")
                    nc.tensor.matmul(rbc[:], onesr_sb[:], rec[:],
                                     start=True, stop=True)
                    at_t = smallp.tile([128, QT], F32, tag="att", name="att")
                    nc.scalar.copy(at_t[:], pvps[:])
                    nc.vector.tensor_tensor(
                        at_all[:, h, qsl], at_t[:], rbc[:],
                        mybir.AluOpType.mult)

        # ================= Phase C =================
        with ExitStack() as cctx:
            outp = cctx.enter_context(tc.tile_pool(name="cout", bufs=4))
            psO = cctx.enter_context(
                tc.tile_pool(name="cpso", bufs=4, space="PSUM"))

            for sc in range(S // 128):
                ssl = bass.ts(sc, 128)
                for nt in range(HID // QT):
                    nsl = bass.ts(nt, QT)
                    ps = psO.tile([128, QT], F32, tag="o", name="o")
                    for g in range(NH):
                        nc.tensor.matmul(
                            ps[:],
                            at_all[:, g, ssl],
                            wo_sb[:, g, nsl],
                            start=(g == 0), stop=(g == NH - 1),
                        )
                    ot = outp.tile([128, QT], F16, tag="ot", name="ot")
                    if nt % 2 == 0:
                        nc.vector.tensor_copy(ot[:], ps[:])
                    else:
                        nc.scalar.copy(ot[:], ps[:])
                    nc.sync.dma_start(out.ap()[ssl, nsl], ot[:])

    nc.compile()
    return nc


def _fp8_split(x):
    hi = x.astype(ml_dtypes.float8_e4m3)
    lo = (x - hi.astype(np.float32)).astype(ml_dtypes.float8_e4m3)
    return (np.ascontiguousarray(hi).view(np.uint8),
            np.ascontiguousarray(lo).view(np.uint8))


def _prep_in_maps(hidden_states, cos, sin, w_qkv, w_o):
    hs = np.ascontiguousarray(np.asarray(hidden_states, dtype=np.float32))
    cos = np.asarray(cos, dtype=np.float32)
    sin = np.asarray(sin, dtype=np.float32)
    w_qkv = np.asarray(w_qkv, dtype=np.float32)
    w_o = np.asarray(w_o, dtype=np.float32)

    wT = np.ascontiguousarray(w_qkv.T) * ALPHA   # (HID, 3*H*D), pre-scaled
    woTf = np.ascontiguousarray(w_o.T)           # (H*D, HID)
    cosT = (np.ascontiguousarray(cos.T) / ALPHA).astype(np.float16)
    sinT = np.ascontiguousarray(sin.T)
    sinS = sinT.copy()
    sinS[:64] = -sinT[:64]
    sinS = (sinS / ALPHA).astype(np.float16)
    tri = np.triu(np.ones((128, 128), np.float16))
    ones = np.ones((128, 1), np.float16)
    onesr = np.ones((1, 128), np.float32)

    h_split = [_fp8_split(np.ascontiguousarray(hs[b].T)) for b in range(B)]
    w_split = []                                 # per head-group hi/lo
    for hg in range(4):
        lo_, hi_ = hg * NH * D, (hg + 1) * NH * D
        w_split.append({
            "wq": _fp8_split(np.ascontiguousarray(wT[:, lo_:hi_])),
            "wk": _fp8_split(np.ascontiguousarray(
                wT[:, H * D + lo_:H * D + hi_])),
            "wv": _fp8_split(np.ascontiguousarray(
                wT[:, 2 * H * D + lo_:2 * H * D + hi_])),
            "wo": np.ascontiguousarray(woTf[lo_:hi_, :]).astype(np.float16),
        })

    in_maps = []
    for c in range(NCORES):
        b, hg = c // 4, c % 4
        ws = w_split[hg]
        in_maps.append({
            "h_hi": h_split[b][0],
            "h_lo": h_split[b][1],
            "wqh": ws["wq"][0], "wql": ws["wq"][1],
            "wkh": ws["wk"][0], "wkl": ws["wk"][1],
            "wvh": ws["wv"][0], "wvl": ws["wv"][1],
            "woT": ws["wo"],
            "cosT": cosT,
            "sinS": sinS,
            "tri": tri,
            "ones": ones,
            "onesr": onesr,
        })
    return in_maps


def kernel(hidden_states, cos, sin, w_qkv, w_o, _trace=False):
    if "nc" not in _CACHED:
        _CACHED["nc"] = _build_nc()
    nc = _CACHED["nc"]
    in_maps = _prep_in_maps(hidden_states, cos, sin, w_qkv, w_o)
    res = run_bass_kernel_spmd(nc, in_maps, core_ids=list(range(NCORES)),
                               trace=_trace)
    _CACHED["last_result"] = res
    out = np.zeros((B, S, HID), np.float32)
    for c in range(NCORES):
        out[c // 4] += res.results[c]["out"]
    return out


# revision 20
# speedup vs baseline: 1.3118x; 1.0420x over previous
"""Trainium2 Bass kernel for AttentionWithRoPE (B=2, S=2048, HID=2048, H=16, D=128).

Sharding (8 cores): tensor-parallel over heads x data-parallel over batch.
Core c handles batch c//4 and heads 4*(c%4) .. 4*(c%4)+4.

Key structure (v2):
  - QKV projection runs as fp8e4m3 DoubleRow matmuls (2 k-chunks per
    instruction, 0.5 cyc/row) with 3-term error compensation: operands are
    split hi/lo on the HOST (x ~ x_hi + x_lo, both fp8) and the product is
    x_hi*w_hi + x_lo*w_hi + x_hi*w_lo (the lo*lo term is ~1e-3 relative and
    dropped). Weights are pre-scaled by 64 on the host so their magnitudes
    sit mid-range in fp8; the scale is undone for free in the RoPE cos/sin
    tables (Q,K) and in the PSUM->SBUF copy (V). Single fused pass over
    hidden produces Q (RoPE'd), K (RoPE'd) and V per s-tile, so hidden is
    loaded once (fp8 hi+lo = half the f32 bytes).
  - Attention per head in scores^T orientation ([k, q]); q/k/v/exp(p) all
    live in SBUF as fp16 (PE rate is the same, DVE gets 2-4x, SBUF/DMA
    halve). exp on ScalarE straight from PSUM with 1/sqrt(D) folded in;
    fully-masked k-blocks skipped; diagonal blocks masked with a 0/1
    upper-tri fp16 tile. The softmax denominator no longer burns PE
    columns per chunk: exp chunks are accumulated into an fp16 E_acc on
    DVE (scalar_tensor_tensor, 4x all-SBUF fp16 mode) and ONE ones-vector
    PE matmul per (head, q-tile) reduces E_acc's 128 partitions.
    Normalization: reciprocal + K=1 PE-matmul broadcast + DVE multiply.
  - Output projection with the core's fp16 w_o column slice; the host sums
    the four partials per batch (the TP reduce).
  - DMA issue order is arranged so the first Q matmul only waits for
    wq_hi + the first hidden chunk (~1.5 MB), not the whole weight set.
"""
import numpy as np
import ml_dtypes
from contextlib import ExitStack

import concourse.bass as bass
import concourse.tile as tile
from concourse import bacc, mybir
from concourse.bass_utils import run_bass_kernel_spmd

B, S, HID = 2, 2048, 2048
H, D = 16, 128
NCORES = 8
NH = 4                 # heads per core
HC = HID // 128        # hid chunks
HCP = HC // 2          # hid chunk pairs (DoubleRow)
AST = 512              # phase-A s-tile width
ANST = S // AST
QT = 512               # phase-B q-tile width
NQT = S // QT
DSCALE = float(D) ** -0.5
ALPHA = 64.0           # host-side weight scale for fp8
F32 = mybir.dt.float32
F32R = mybir.dt.float32r
F16 = mybir.dt.float16
F8 = mybir.dt.float8e4
DR = mybir.MatmulPerfMode.DoubleRow

_CACHED = {}


def _build_nc():
    nc = bacc.Bacc("TRN2", target_bir_lowering=False, debug=False,
                   num_devices=NCORES)
    h_hi = nc.dram_tensor("h_hi", [HID, S], F8, kind="ExternalInput")
    h_lo = nc.dram_tensor("h_lo", [HID, S], F8, kind="ExternalInput")
    wqh = nc.dram_tensor("wqh", [HID, NH * D], F8, kind="ExternalInput")
    wql = nc.dram_tensor("wql", [HID, NH * D], F8, kind="ExternalInput")
    wkh = nc.dram_tensor("wkh", [HID, NH * D], F8, kind="ExternalInput")
    wkl = nc.dram_tensor("wkl", [HID, NH * D], F8, kind="ExternalInput")
    wvh = nc.dram_tensor("wvh", [HID, NH * D], F8, kind="ExternalInput")
    wvl = nc.dram_tensor("wvl", [HID, NH * D], F8, kind="ExternalInput")
    woT = nc.dram_tensor("woT", [NH * D, HID], F16, kind="ExternalInput")
    cosT = nc.dram_tensor("cosT", [D, S], F16, kind="ExternalInput")
    sinS = nc.dram_tensor("sinS", [D, S], F16, kind="ExternalInput")
    tri = nc.dram_tensor("tri", [128, 128], F16, kind="ExternalInput")
    ones = nc.dram_tensor("ones", [128, 128], F16, kind="ExternalInput")
    out = nc.dram_tensor("out", [S, HID], F16, kind="ExternalOutput")

    hh_r = h_hi.ap().rearrange("(hc p) s -> p hc s", p=128)
    hl_r = h_lo.ap().rearrange("(hc p) s -> p hc s", p=128)
    w_r = {w.name: w.ap().rearrange("(hc p) m -> p hc m", p=128)
           for w in (wqh, wql, wkh, wkl, wvh, wvl)}
    woT_r = woT.ap().rearrange("(g p) n -> p g n", p=128)

    with tile.TileContext(nc) as tc, ExitStack() as ctx:
        # ---- small constants ----
        constp = ctx.enter_context(tc.tile_pool(name="const", bufs=1))
        tri_sb = constp.tile([128, 128], F16, tag="tri", name="tri")
        nc.sync.dma_start(tri_sb[:], tri.ap())
        # ones MATRIX: colsum matmul with this lhsT writes the softmax
        # denominator broadcast to all 128 partitions in one shot
        ones_sb = constp.tile([128, 128], F16, tag="ones", name="ones")
        nc.sync.dma_start(ones_sb[:], ones.ap())

        # Q^T/K^T stay resident in SBUF through attention (fp16)
        qkp = ctx.enter_context(tc.tile_pool(name="qk", bufs=1))
        qsb = qkp.tile([128, NH, S], F16, tag="qsb", name="qsb")
        ksb = qkp.tile([128, NH, S], F16, tag="ksb", name="ksb")
        vp = ctx.enter_context(tc.tile_pool(name="vp", bufs=1))
        v_sb = vp.tile([128, S // 128, NH * D], F16, tag="vsb", name="vsb")

        # fp8 hi/lo weights, all resident
        wp = ctx.enter_context(tc.tile_pool(name="w", bufs=1))
        wsb = {name: wp.tile([128, HC, NH * D], F8, tag=name, name=name)
               for name in ("wqh", "wql", "wkh", "wkl", "wvh", "wvl")}

        # ================= Phase A: fused QKV =================
        with ExitStack() as astack:
            hpool = astack.enter_context(tc.tile_pool(name="ah", bufs=3))
            cspool = astack.enter_context(tc.tile_pool(name="acs", bufs=2))
            ropep = astack.enter_context(tc.tile_pool(name="arope", bufs=1))
            psA = astack.enter_context(
                tc.tile_pool(name="apsqk", bufs=5, space="PSUM"))
            psV = astack.enter_context(
                tc.tile_pool(name="apsv", bufs=3, space="PSUM"))

            def load_tile(st):
                sl = bass.ts(st, AST)
                hh = hpool.tile([128, HC, AST], F8, tag="hh", name="hh")
                hl = hpool.tile([128, HC, AST], F8, tag="hl", name="hl")
                cs_t = cspool.tile([128, AST], F16, tag="cs", name="cs")
                ss_t = cspool.tile([128, AST], F16, tag="ss", name="ss")
                if st == 0:
                    # ordered so the first matmuls' inputs land first
                    nc.sync.dma_start(wsb["wqh"][:], w_r["wqh"])
                    for c in range(4):
                        nc.sync.dma_start(hh[:, 4 * c:4 * c + 4, :],
                                          hh_r[:, 4 * c:4 * c + 4, sl])
                    nc.sync.dma_start(wsb["wql"][:], w_r["wql"])
                    nc.sync.dma_start(cs_t[:], cosT.ap()[:, sl])
                    nc.sync.dma_start(ss_t[:], sinS.ap()[:, sl])
                    for c in range(4):
                        nc.sync.dma_start(hl[:, 4 * c:4 * c + 4, :],
                                          hl_r[:, 4 * c:4 * c + 4, sl])
                    for name in ("wkh", "wkl", "wvh", "wvl"):
                        nc.sync.dma_start(wsb[name][:], w_r[name])
                else:
                    for c in range(4):
                        nc.sync.dma_start(hh[:, 4 * c:4 * c + 4, :],
                                          hh_r[:, 4 * c:4 * c + 4, sl])
                    nc.sync.dma_start(cs_t[:], cosT.ap()[:, sl])
                    nc.sync.dma_start(ss_t[:], sinS.ap()[:, sl])
                    for c in range(4):
                        nc.sync.dma_start(hl[:, 4 * c:4 * c + 4, :],
                                          hl_r[:, 4 * c:4 * c + 4, sl])
                return hh, hl, cs_t, ss_t

            for st in range(ANST):
                sl = bass.ts(st, AST)
                hh, hl, cs_t, ss_t = load_tile(st)

                for wn, dsb in (("wq", qsb), ("wk", ksb)):
                    whi, wlo = wsb[wn + "h"], wsb[wn + "l"]
                    for h in range(NH):
                        hsl = slice(h * D, (h + 1) * D)
                        ps = psA.tile([128, AST], F32, tag="psqk",
                                      name="psqk")
                        k = 0
                        for wt, ht in ((whi, hh), (wlo, hh), (whi, hl)):
                            for p in range(HCP):
                                nc.tensor.matmul(
                                    ps[:],
                                    wt[:, 2 * p:2 * p + 2, hsl],
                                    ht[:, 2 * p:2 * p + 2, :],
                                    start=(k == 0), stop=(k == 3 * HCP - 1),
                                    perf_mode=DR,
                                )
                                k += 1
                        # RoPE: out = x*cos + shift(x)*sin_signed, with the
                        # 1/ALPHA weight descale folded into the host tables.
                        # PSUM-reading muls must stay on DVE (GPSIMD cannot
                        # access PSUM); the all-SBUF fp16 add runs on the
                        # idle Pool engine.
                        tsin = ropep.tile([128, AST], F16, tag="tsin",
                                          name="tsin")
                        nc.vector.tensor_tensor(
                            tsin[0:64, :], ps[64:128, :], ss_t[0:64, :],
                            mybir.AluOpType.mult)
                        nc.vector.tensor_tensor(
                            tsin[64:128, :], ps[0:64, :], ss_t[64:128, :],
                            mybir.AluOpType.mult)
                        tcos = ropep.tile([128, AST], F16, tag="tcos",
                                          name="tcos")
                        nc.vector.tensor_tensor(
                            tcos[:], ps[:], cs_t[:], mybir.AluOpType.mult)
                        nc.gpsimd.tensor_tensor(
                            dsb[:, h, sl], tcos[:], tsin[:],
                            mybir.AluOpType.add)

                # V: natural orientation, 1/ALPHA descale in the PSUM copy
                for sc in range(AST // 128):
                    scl = slice(sc * 128, (sc + 1) * 128)
                    ps = psV.tile([128, NH * D], F32, tag="psv", name="psv")
                    k = 0
                    for wt, ht in ((wsb["wvh"], hh), (wsb["wvl"], hh),
                                   (wsb["wvh"], hl)):
                        for p in range(HCP):
                            nc.tensor.matmul(
                                ps[:],
                                ht[:, 2 * p:2 * p + 2, scl],
                                wt[:, 2 * p:2 * p + 2, :],
                                start=(k == 0), stop=(k == 3 * HCP - 1),
                                perf_mode=DR,
                            )
                            k += 1
                    nc.scalar.mul(
                        v_sb[:, st * (AST // 128) + sc, :], ps[:],
                        1.0 / ALPHA)

        # w_o prefetches during phase B
        wop = ctx.enter_context(tc.tile_pool(name="cwo", bufs=1))
        wo_sb = wop.tile([128, NH, HID], F16, tag="wo", name="wo")
        for g in range(NH):
            nc.sync.dma_start(wo_sb[:, g, :], woT_r[:, g, :])
        atp = ctx.enter_context(tc.tile_pool(name="at", bufs=1))
        at_all = atp.tile([128, NH, S], F16, tag="at", name="at")

        # ========= Phase B + C, software-pipelined =========
        # qt-outer / h-inner; phase C (s,n)-units for qt-block j are emitted
        # interleaved with qt-block j+1's attention iterations, so the
        # Act-bound softmax stretches keep the PE fed with output-projection
        # matmuls. The C PSUM allocations share the PV pool ring.
        with ExitStack() as bctx:
            expp = bctx.enter_context(tc.tile_pool(name="bexp", bufs=8))
            eaccp = bctx.enter_context(tc.tile_pool(name="beacc", bufs=2))
            smallp = bctx.enter_context(tc.tile_pool(name="bsmall", bufs=3))
            outp = bctx.enter_context(tc.tile_pool(name="cout", bufs=4))
            psS = bctx.enter_context(
                tc.tile_pool(name="bpss", bufs=2, space="PSUM"))
            psPVO = bctx.enter_context(
                tc.tile_pool(name="bpspv", bufs=3, space="PSUM"))
            psCS = bctx.enter_context(
                tc.tile_pool(name="bpscs", bufs=1, space="PSUM"))

            cunits = [(sc, nt) for sc in range(S // 128)
                      for nt in range(HID // QT)]
            cpos = 0

            def emit_c_unit(sc, nt, flip):
                ssl = bass.ts(sc, 128)
                nsl = bass.ts(nt, QT)
                ps = psPVO.tile([128, QT], F32, tag="pv", name="o")
                for g in range(NH):
                    nc.tensor.matmul(
                        ps[:],
                        at_all[:, g, ssl],
                        wo_sb[:, g, nsl],
                        start=(g == 0), stop=(g == NH - 1),
                    )
                ot = outp.tile([128, QT], F16, tag="ot", name="ot")
                if flip:
                    nc.vector.tensor_copy(ot[:], ps[:])
                else:
                    nc.scalar.copy(ot[:], ps[:])
                nc.sync.dma_start(out.ap()[ssl, nsl], ot[:])

            for qt in range(NQT):
                for h in range(NH):
                    nallow = (QT // 128) * qt + (QT // 128)
                    qsl = bass.ts(qt, QT)
                    pvps = psPVO.tile([128, QT], F32, tag="pv", name="pv")
                    csps = psCS.tile([128, QT], F32, tag="cs", name="cs")
                    # split exp accumulators: DVE takes off-diagonal chunks,
                    # Pool the diagonal ones (diag chunks start at lo=0 so
                    # epool always initializes full-width)
                    edve = eaccp.tile([128, QT], F16, tag="ea", name="ea")
                    epool = eaccp.tile([128, QT], F16, tag="ep", name="ep")
                    dve_init = pool_init = False

                    ngrp = (nallow + 1) // 2
                    egrp = []
                    for g in range(ngrp):
                        k0 = 2 * g
                        nk = min(2, nallow - k0)
                        sps = psS.tile([128, 2, QT], F32, tag="s", name="s")
                        eb = expp.tile([128, 2, QT], F16, tag="e", name="e")
                        egrp.append(eb)
                        for i in range(nk):
                            kc = k0 + i
                            lo = max(0, 128 * (kc - 4 * qt))
                            nc.tensor.matmul(
                                sps[:, i, lo:QT],
                                ksb[:, h, kc * 128:(kc + 1) * 128],
                                qsb[:, h, qt * QT + lo:(qt + 1) * QT],
                                start=True, stop=True,
                            )
                        j0 = k0 - 4 * qt
                        if j0 + nk - 1 < 0:
                            nc.scalar.activation(
                                eb[:, 0:nk, :], sps[:, 0:nk, :],
                                mybir.ActivationFunctionType.Exp,
                                scale=DSCALE)
                        else:
                            for i in range(nk):
                                kc = k0 + i
                                j = kc - 4 * qt
                                lo = max(0, 128 * j)
                                nc.scalar.activation(
                                    eb[:, i, lo:QT], sps[:, i, lo:QT],
                                    mybir.ActivationFunctionType.Exp,
                                    scale=DSCALE)
                                if j >= 0:
                                    nc.vector.tensor_tensor(
                                        eb[:, i, lo:lo + 128],
                                        eb[:, i, lo:lo + 128],
                                        tri_sb[:],
                                        mybir.AluOpType.mult)
                        for i in range(nk):
                            kc = k0 + i
                            lo = max(0, 128 * (kc - 4 * qt))
                            if kc - 4 * qt >= 0:      # diagonal: Pool engine
                                if not pool_init:
                                    nc.gpsimd.tensor_copy(
                                        epool[:, lo:QT], eb[:, i, lo:QT])
                                    pool_init = True
                                else:
                                    nc.gpsimd.tensor_tensor(
                                        epool[:, lo:QT], eb[:, i, lo:QT],
                                        epool[:, lo:QT], mybir.AluOpType.add)
                            else:
                                if not dve_init:
                                    nc.vector.tensor_copy(
                                        edve[:, lo:QT], eb[:, i, lo:QT])
                                    dve_init = True
                                else:
                                    nc.vector.tensor_tensor(
                                        edve[:, lo:QT], eb[:, i, lo:QT],
                                        edve[:, lo:QT], mybir.AluOpType.add)

                    # PV accumulation over allowed chunks
                    for kc in range(nallow):
                        j = kc - 4 * qt
                        lo = max(0, 128 * j)
                        eb = egrp[kc // 2]
                        i = kc % 2
                        nc.tensor.matmul(
                            pvps[:, lo:QT],
                            v_sb[:, kc, h * D:(h + 1) * D],
                            eb[:, i, lo:QT],
                            start=(kc == 0), stop=(kc == nallow - 1),
                            skip_group_check=True,
                        )

                    # denominator, broadcast to all partitions by a ones-
                    # matrix lhsT, accumulated over the two accumulators
                    accs = ([edve] if dve_init else []) + [epool]
                    for ai, acc in enumerate(accs):
                        nc.tensor.matmul(csps[:], ones_sb[:], acc[:],
                                         start=(ai == 0),
                                         stop=(ai == len(accs) - 1),
                                         skip_group_check=True)
                    rec = smallp.tile([128, QT], F16, tag="rec", name="rec")
                    with nc.allow_low_precision(
                            reason="softmax denom reciprocal to fp16"):
                        nc.vector.reciprocal(rec[:], csps[:])
                    at_t = smallp.tile([128, QT], F16, tag="att", name="att")
                    nc.vector.tensor_copy(at_t[:], pvps[:])
                    nc.vector.tensor_tensor(
                        at_all[:, h, qsl], at_t[:], rec[:],
                        mybir.AluOpType.mult)

                    # phase-C units of the previous qt-block
                    if qt > 0:
                        for u in range(4):
                            sc, nt = cunits[cpos]
                            emit_c_unit(sc, nt, cpos % 2 == 0)
                            cpos += 1

            # remaining phase-C units (last qt-block)
            while cpos < len(cunits):
                sc, nt = cunits[cpos]
                emit_c_unit(sc, nt, cpos % 2 == 0)
                cpos += 1

    nc.compile()
    return nc


def _fp8_split(x):
    hi = x.astype(ml_dtypes.float8_e4m3)
    lo = (x - hi.astype(np.float32)).astype(ml_dtypes.float8_e4m3)
    return (np.ascontiguousarray(hi).view(np.uint8),
            np.ascontiguousarray(lo).view(np.uint8))


def _prep_in_maps(hidden_states, cos, sin, w_qkv, w_o):
    hs = np.ascontiguousarray(np.asarray(hidden_states, dtype=np.float32))
    cos = np.asarray(cos, dtype=np.float32)
    sin = np.asarray(sin, dtype=np.float32)
    w_qkv = np.asarray(w_qkv, dtype=np.float32)
    w_o = np.asarray(w_o, dtype=np.float32)

    wT = np.ascontiguousarray(w_qkv.T) * ALPHA   # (HID, 3*H*D), pre-scaled
    woTf = np.ascontiguousarray(w_o.T)           # (H*D, HID)
    cosT = (np.ascontiguousarray(cos.T) / ALPHA).astype(np.float16)
    sinT = np.ascontiguousarray(sin.T)
    sinS = sinT.copy()
    sinS[:64] = -sinT[:64]
    sinS = (sinS / ALPHA).astype(np.float16)
    tri = np.triu(np.ones((128, 128), np.float16))
    ones = np.ones((128, 128), np.float16)

    h_split = [_fp8_split(np.ascontiguousarray(hs[b].T)) for b in range(B)]
    w_split = []                                 # per head-group hi/lo
    for hg in range(4):
        lo_, hi_ = hg * NH * D, (hg + 1) * NH * D
        w_split.append({
            "wq": _fp8_split(np.ascontiguousarray(wT[:, lo_:hi_])),
            "wk": _fp8_split(np.ascontiguousarray(
                wT[:, H * D + lo_:H * D + hi_])),
            "wv": _fp8_split(np.ascontiguousarray(
                wT[:, 2 * H * D + lo_:2 * H * D + hi_])),
            "wo": np.ascontiguousarray(woTf[lo_:hi_, :]).astype(np.float16),
        })

    in_maps = []
    for c in range(NCORES):
        b, hg = c // 4, c % 4
        ws = w_split[hg]
        in_maps.append({
            "h_hi": h_split[b][0],
            "h_lo": h_split[b][1],
            "wqh": ws["wq"][0], "wql": ws["wq"][1],
            "wkh": ws["wk"][0], "wkl": ws["wk"][1],
            "wvh": ws["wv"][0], "wvl": ws["wv"][1],
            "woT": ws["wo"],
            "cosT": cosT,
            "sinS": sinS,
            "tri": tri,
            "ones": ones,
        })
    return in_maps


def kernel(hidden_states, cos, sin, w_qkv, w_o, _trace=False):
    if "nc" not in _CACHED:
        _CACHED["nc"] = _build_nc()
    nc = _CACHED["nc"]
    in_maps = _prep_in_maps(hidden_states, cos, sin, w_qkv, w_o)
    res = run_bass_kernel_spmd(nc, in_maps, core_ids=list(range(NCORES)),
                               trace=_trace)
    _CACHED["last_result"] = res
    out = np.zeros((B, S, HID), np.float32)
    for c in range(NCORES):
        out[c // 4] += res.results[c]["out"]
    return out
